# revision 23
# baseline (speedup 1.0000x reference)
"""Multi-head self-attention (RoPE, causal) on 8 trn2 NeuronCores.

Sharding: core c -> batch b = c // 4, head group g = c % 4 (4 heads each).
Each core:
  - projects Q,K,V for its batch / its 4 heads (token-major), applies RoPE
    via a gathered cos/sin table, transposes Q,K to [dk, S] layout,
  - computes scores^T = K^T-chunk x Q (k on partitions), exp on ACT,
    then out^T[d, q] accumulated fully in PSUM via V'-stationary matmuls
    where V' carries 64 appended ones-columns so PSUM rows 64:128 hold the
    softmax sums pre-broadcast; normalization is then just an ACT
    partition-shift copy + fast reciprocal + one multiply per q-block,
  - exchanges the head-sharded attention output with its batch peers via
    an 8-way AllToAll per head pair (cross-batch blocks duplicated as
    padding so the program stays SPMD-uniform), gathers its q-slice rows
    with a per-core index table, and runs the output projection in two
    passes so the first AllToAll's data is consumed during attention.
Host only reshapes/transposes/casts inputs and concatenates outputs.
"""

import sys

for _p in ("/opt/trn_rl_repo",):
    if _p not in sys.path:
        sys.path.append(_p)

import numpy as np
import ml_dtypes

import concourse.bass as bass
import concourse.mybir as mybir
import concourse.tile as tile
from concourse import bacc
from concourse.bass import ds, ts
from concourse.bass_utils import run_bass_kernel_spmd
from concourse.masks import make_identity

BF16 = mybir.dt.bfloat16
F32 = mybir.dt.float32
I32 = mybir.dt.int32

B, S, D = 2, 2048, 1024
H, DK = 16, 64
THETA = 10000.0
MAXPOS = 2048
N_CORES = 8
GROUPS = 4          # head groups (cores) per batch
HPC = H // GROUPS   # heads per core = 4
QKV_COLS = 3 * HPC * DK        # 768 per-core projection width
QK_COLS = 2 * HPC * DK         # 512 (Q then K)
NSC = S // 128                 # 16 token chunks
NQC = S // 512                 # 4 q column-chunks
QSLICE = S // GROUPS           # 512 output tokens per core
MUL = mybir.AluOpType.mult
ADD = mybir.AluOpType.add
SUB = mybir.AluOpType.subtract
COPY_F = mybir.ActivationFunctionType.Copy
EXP_F = mybir.ActivationFunctionType.Exp


def _build():
    nc = bacc.Bacc("TRN2", num_devices=N_CORES)

    xT = nc.dram_tensor("xT", [D, S], BF16, kind="ExternalInput")
    wqkvT = nc.dram_tensor("wqkvT", [D, QKV_COLS], BF16, kind="ExternalInput")
    woT = nc.dram_tensor("woT", [D, D], BF16, kind="ExternalInput")
    cstab = nc.dram_tensor("cstab", [MAXPOS, 2 * DK], F32, kind="ExternalInput")
    pos = nc.dram_tensor("pos", [128, NSC], I32, kind="ExternalInput")
    tri = nc.dram_tensor("tri", [128, 128], BF16, kind="ExternalInput")
    wsel = nc.dram_tensor("wsel", [128, GROUPS], I32, kind="ExternalInput")
    finT = nc.dram_tensor("finT", [D, QSLICE], BF16, kind="ExternalOutput")

    with tile.TileContext(nc) as tc:
        with (
            tc.tile_pool(name="const", bufs=1) as constp,
            tc.tile_pool(name="wts", bufs=1) as wtsp,
            tc.tile_pool(name="seq", bufs=1) as seqp,
            tc.tile_pool(name="attp", bufs=1) as attp,
            tc.tile_pool(name="dram", bufs=1, space="DRAM") as dramp,
        ):
            # ---------------- early DMAs: only what phase 1 needs ----------
            wt = wtsp.tile([128, 8, QKV_COLS], BF16)       # [dchunk][768]
            for k in range(8):
                nc.sync.dma_start(out=wt[:, k, 0:512],
                                  in_=wqkvT[ts(k, 128), 0:512])

            pidx = constp.tile([128, NSC], I32)
            nc.sync.dma_start(out=pidx[:], in_=pos[:])
            tri_t = constp.tile([128, 128], BF16)
            nc.sync.dma_start(out=tri_t[:], in_=tri[:])
            wsel_sb = constp.tile([128, GROUPS], I32)
            nc.sync.dma_start(out=wsel_sb[:], in_=wsel[:])
            ident = constp.tile([128, 128], BF16)
            make_identity(nc, ident[:])

            # persistent per-core tensors
            qt = seqp.tile([128, 2, S], BF16)   # Q^T  [pair, dk(2x64), q]
            kt = seqp.tile([128, 2, S], BF16)   # K^T
            # V + 64 ones cols per head: [k-token][chunk][head][v64|ones64]
            vv = seqp.tile([128, NSC, HPC, 2 * DK], BF16)
            nc.vector.memset(vv[:, :, :, DK:], 1.0)
            attT = attp.tile([128, 2, S], BF16)
            rwo = attp.tile([128, 8, QSLICE], BF16)
            fe = attp.tile([128, 8, QSLICE], F32)
            wo = wtsp.tile([128, 8, D], BF16)   # loaded late (see below)

            # ------------- projection + RoPE + transposes (scoped x) -------
            with (
                tc.tile_pool(name="xap", bufs=1) as xap,
                tc.tile_pool(name="ropet", bufs=2) as ropet,
                tc.tile_pool(name="pbig", bufs=2, space="PSUM") as pbig,
                tc.tile_pool(name="psmall", bufs=2, space="PSUM") as psmall,
            ):
                xa = xap.tile([128, 8, S], BF16)           # resident x^T
                for k in range(8):
                    nc.sync.dma_start(
                        out=xa[:, k, 0:512], in_=xT[ts(k, 128), 0:512])
                for k in range(8):
                    nc.sync.dma_start(out=wt[:, k, 512:768],
                                      in_=wqkvT[ts(k, 128), 512:768])
                for q in range(1, 4):
                    for k in range(8):
                        nc.sync.dma_start(
                            out=xa[:, k, ts(q, 512)],
                            in_=xT[ts(k, 128), ts(q, 512)])
                cs = xap.tile([128, NSC, 2 * DK], F32)     # cos/sin gather
                for c in range(NSC):
                    nc.gpsimd.indirect_dma_start(
                        out=cs[:, c, :],
                        out_offset=None,
                        in_=cstab[:],
                        in_offset=bass.IndirectOffsetOnAxis(
                            ap=pidx[:, c:c + 1], axis=0),
                    )

                for sc in range(NSC):
                    ps = pbig.tile([128, QKV_COLS], F32, space="PSUM",
                                   tag="big")
                    for k in range(8):
                        nc.tensor.matmul(
                            ps[:, 0:512], lhsT=xa[:, k, ts(sc, 128)],
                            rhs=wt[:, k, 0:512],
                            start=(k == 0), stop=(k == 7),
                        )
                    for k in range(8):
                        nc.tensor.matmul(
                            ps[:, 512:768], lhsT=xa[:, k, ts(sc, 128)],
                            rhs=wt[:, k, 512:768],
                            start=(k == 0), stop=(k == 7),
                        )

                    # RoPE over the Q,K halves (cols 0:512), 8 blocks of 64
                    def qk_ap(off):
                        a = ps[:]
                        return bass.AP(a.tensor, a.offset + off,
                                       [a.ap[0], [DK, 2 * HPC], [2, DK // 2]])

                    def cs_ap(off):
                        a = cs[:, sc, :]
                        return bass.AP(a.tensor, a.offset + off,
                                       [a.ap[0], [0, 2 * HPC], [2, DK // 2]])

                    t1 = ropet.tile([128, 2 * HPC, DK // 2], F32, tag="t1")
                    t2 = ropet.tile([128, 2 * HPC, DK // 2], F32, tag="t2")
                    t3 = ropet.tile([128, 2 * HPC, DK // 2], F32, tag="t3")
                    t4 = ropet.tile([128, 2 * HPC, DK // 2], F32, tag="t4")
                    roped = ropet.tile([128, QK_COLS], BF16, tag="roped")

                    def roped_ap(off):
                        a = roped[:]
                        return bass.AP(a.tensor, a.offset + off,
                                       [a.ap[0], [DK, 2 * HPC], [2, DK // 2]])

                    nc.vector.tensor_tensor(t1[:], qk_ap(0), cs_ap(0), MUL)
                    nc.vector.tensor_tensor(t2[:], qk_ap(1), cs_ap(DK), MUL)
                    nc.vector.tensor_tensor(roped_ap(0), t1[:], t2[:], SUB)
                    nc.vector.tensor_tensor(t3[:], qk_ap(0), cs_ap(DK), MUL)
                    nc.vector.tensor_tensor(t4[:], qk_ap(1), cs_ap(0), MUL)
                    nc.vector.tensor_tensor(roped_ap(1), t3[:], t4[:], ADD)

                    # V columns (ones cols already set) -- on ACT
                    nc.scalar.activation(
                        vv[:, sc, :, 0:DK],
                        ps[:, 512:768].rearrange("p (h e) -> p h e", h=HPC),
                        COPY_F,
                    )

                    # transpose the 4 roped q/k 128-col blocks -> qt/kt
                    for t in range(4):
                        tp = psmall.tile([128, 128], BF16, space="PSUM",
                                         tag="small")
                        nc.tensor.transpose(tp[:], roped[:, ts(t, 128)],
                                            ident[:])
                        dst = qt if t < 2 else kt
                        nc.scalar.activation(dst[:, t % 2, ts(sc, 128)],
                                             tp[:], COPY_F)

                # late weight load: queued behind phase-1 DMAs on purpose
                for k in range(8):
                    nc.sync.dma_start(out=wo[:, k, :], in_=woT[ts(k, 128), :])

            # ------------- attention (per head) + 8-way A2A per head pair --
            agin = [dramp.tile([8 * 128, QSLICE], BF16, name=f"agin{p}")
                    for p in range(2)]
            agout = [dramp.tile([8 * 128, QSLICE], BF16, name=f"agout{p}")
                     for p in range(2)]

            with (
                tc.tile_pool(name="etp", bufs=3) as etp,
                tc.tile_pool(name="rsbp", bufs=2) as rsbp,
                tc.tile_pool(name="scoresp", bufs=2, space="PSUM") as scoresp,
                tc.tile_pool(name="vaccp", bufs=4, space="PSUM") as vaccp,
            ):
                def emit_sblock(h, j, et, qh):
                    # scores + one exp for k-chunk j, 1024-col window qh
                    pr, hf = h // 2, (h % 2) * DK
                    a0 = max(1024 * qh, 128 * j)
                    sp = scoresp.tile([128, 1024], F32, space="PSUM",
                                      tag="sc", name=f"sp{h}_{j}_{qh}")
                    for qq in (1024 * qh, 1024 * qh + 512):
                        a = max(a0, qq)
                        w = qq + 512 - a
                        if w <= 0:
                            continue
                        nc.tensor.matmul(
                            sp[:, ds(a - 1024 * qh, w)],
                            lhsT=kt[ds(hf, DK), pr, ts(j, 128)],
                            rhs=qt[ds(hf, DK), pr, ds(a, w)],
                            start=True, stop=True,
                        )
                    nc.scalar.activation(
                        et[:, ds(a0, 1024 * (qh + 1) - a0)],
                        sp[:, ds(a0 - 1024 * qh, 1024 * (qh + 1) - a0)],
                        EXP_F)
                    if qh == j // 8:   # diagonal block: mask q < k to 0
                        nc.vector.tensor_tensor(
                            et[:, ts(j, 128)], et[:, ts(j, 128)], tri_t[:],
                            MUL)

                def emit_vblock(h, j, et, vacc, qb):
                    a = max(512 * qb, 128 * j)
                    w = 512 * (qb + 1) - a
                    nc.tensor.matmul(
                        vacc[qb][:, ds(a - 512 * qb, w)],
                        lhsT=vv[:, j, h, :],
                        rhs=et[:, ds(a, w)],
                        start=(j == 0), stop=(j == 4 * qb + 3),
                    )

                def emit_normalize(h, vacc):
                    pr, hf = h // 2, (h % 2) * DK
                    for qc in range(NQC):
                        rsb = rsbp.tile([DK, 512], F32, tag="rsb")
                        nc.vector.tensor_copy(rsb[:], vacc[qc][ds(DK, DK), :])
                        nc.vector.reciprocal_approx_fast(rsb[:], rsb[:])
                        nc.vector.tensor_tensor(
                            attT[ds(hf, DK), pr, ts(qc, 512)],
                            vacc[qc][0:DK, :], rsb[:], MUL)
                        if h % 2 == 1:   # pair slice done -> stage for A2A
                            p = h // 2
                            for bh in range(2):
                                nc.sync.dma_start(
                                    out=agin[p][ds((4 * bh + qc) * 128, 128), :],
                                    in_=attT[:, p, ts(qc, 512)],
                                )

                def emit_a2a(p):
                    nc.gpsimd.collective_compute(
                        "AllToAll", mybir.AluOpType.bypass,
                        ins=[agin[p][:]], outs=[agout[p][:]],
                        replica_groups=[[0, 1, 2, 3, 4, 5, 6, 7]],
                    )
                    for j in range(GROUPS):
                        nc.gpsimd.indirect_dma_start(
                            out=rwo[:, 2 * j + p, :],
                            out_offset=None,
                            in_=agout[p][:],
                            in_offset=bass.IndirectOffsetOnAxis(
                                ap=wsel_sb[:, j:j + 1], axis=0),
                        )

                def emit_proj1(ec):
                    # shares the scores psum ring (same shape, half used)
                    fp = scoresp.tile([128, 1024], F32, space="PSUM",
                                      tag="sc", name=f"fp{ec}")
                    for i, dp in enumerate((0, 2, 4, 6)):
                        nc.tensor.matmul(
                            fp[:, 0:QSLICE], lhsT=wo[:, dp, ts(ec, 128)],
                            rhs=rwo[:, dp, :],
                            start=(i == 0), stop=(i == 3),
                        )
                    nc.vector.tensor_copy(fe[:, ec, :], fp[:, 0:QSLICE])

                for h in range(HPC):
                    vacc = [vaccp.tile([128, 512], F32, space="PSUM",
                                       tag="va", name=f"va{h}_{q}")
                            for q in range(NQC)]
                    ets = {}
                    for j in range(NSC):
                        et = etp.tile([128, S], BF16, tag="exp",
                                      name=f"et{h}_{j}")
                        ets[j] = et
                        # interleave: scores/exp for (j, qh) two chunks ahead
                        # of the V-matmuls for (j-2, qb) so ACT stays ahead
                        jv = j - 2
                        vqbs = (list(range(jv // 4, NQC)) if jv >= 0 else [])
                        for qh in range(j // 8, 2):
                            emit_sblock(h, j, et, qh)
                            while vqbs and (vqbs[0] <= 2 * qh + 1 or qh == 1):
                                emit_vblock(h, jv, ets[jv], vacc,
                                            vqbs.pop(0))
                        if h == 3 and j >= 8:   # interleave proj pass 1
                            emit_proj1(j - 8)
                        ets.pop(j - 2, None)
                    for jv in (NSC - 2, NSC - 1):
                        for qb in range(jv // 4, NQC):
                            emit_vblock(h, jv, ets[jv], vacc, qb)
                    emit_normalize(h, vacc)
                    if h % 2 == 1:
                        emit_a2a(h // 2)

            # ------------- output projection pass 2 ------------------------
            with (
                tc.tile_pool(name="finp", bufs=2) as finp,
                tc.tile_pool(name="proj2p", bufs=2, space="PSUM") as proj2p,
            ):
                for ec in range(8):
                    fp = proj2p.tile([128, QSLICE], F32, space="PSUM",
                                     tag="p2")
                    for i, dp in enumerate((1, 3, 5, 7)):
                        nc.tensor.matmul(
                            fp[:], lhsT=wo[:, dp, ts(ec, 128)],
                            rhs=rwo[:, dp, :],
                            start=(i == 0), stop=(i == 3),
                        )
                    fin_sb = finp.tile([128, QSLICE], BF16, tag="fin")
                    nc.vector.tensor_tensor(fin_sb[:], fe[:, ec, :], fp[:],
                                            ADD)
                    nc.sync.dma_start(out=finT[ts(ec, 128), :], in_=fin_sb[:])

    nc.compile()
    return nc


def _host_prep(x, token_positions, W_qkv, W_o):
    bf16 = ml_dtypes.bfloat16
    xT = np.ascontiguousarray(np.transpose(x, (0, 2, 1))).astype(bf16)  # [B,D,S]

    # per-group W_qkv^T slices (Q rows pre-scaled by 1/sqrt(dk))
    wq = W_qkv[0 * D:1 * D] * np.float32(1.0 / np.sqrt(DK))
    wk = W_qkv[1 * D:2 * D]
    wv = W_qkv[2 * D:3 * D]
    wslices = []
    for g in range(GROUPS):
        rows = slice(g * HPC * DK, (g + 1) * HPC * DK)
        wsl = np.concatenate([wq[rows], wk[rows], wv[rows]], axis=0)  # [768, D]
        wslices.append(np.ascontiguousarray(wsl.T).astype(bf16))      # [D, 768]

    woT = np.ascontiguousarray(W_o.T).astype(bf16)                    # [D, D]

    idx = np.arange(DK // 2, dtype=np.float64)
    freqs = 1.0 / (THETA ** (2.0 * idx / DK))
    ang = np.arange(MAXPOS, dtype=np.float64)[:, None] * freqs[None, :]
    cstab = np.zeros((MAXPOS, 2 * DK), dtype=np.float32)
    cstab[:, 0:DK:2] = np.cos(ang)
    cstab[:, 1:DK:2] = np.cos(ang)
    cstab[:, DK::2] = np.sin(ang)
    cstab[:, DK + 1::2] = np.sin(ang)

    tri = (np.arange(128)[None, :] >= np.arange(128)[:, None]).astype(bf16)

    # pos transposed host-side: pos[r, c] = positions[128*c + r]
    posi = np.asarray(token_positions).astype(np.int32).reshape(B, NSC, 128)
    posi = np.ascontiguousarray(np.transpose(posi, (0, 2, 1)))  # [B,128,NSC]

    # wsel[r, j] = row of agout[h] holding peer (b, j)'s dims for my slice
    rr = np.arange(128)
    in_maps = []
    for c in range(N_CORES):
        b = c // GROUPS
        wsel = (512 * b + 128 * np.arange(GROUPS)[None, :]
                + rr[:, None]).astype(np.int32)
        in_maps.append({
            "xT": np.asarray(xT[b]),
            "wqkvT": wslices[c % GROUPS],
            "woT": woT,
            "cstab": cstab,
            "pos": posi[b],
            "tri": tri,
            "wsel": wsel,
        })
    return in_maps


def _assemble(results):
    out = np.empty((B, S, D), dtype=np.float32)
    for b in range(B):
        fullT = np.concatenate(
            [results[b * GROUPS + g]["finT"].astype(np.float32)
             for g in range(GROUPS)], axis=1)
        out[b] = fullT.T
    return out


_NC_CACHE = {}


def run(inputs, trace=False, **kw):
    if "nc" not in _NC_CACHE:
        _NC_CACHE["nc"] = _build()
    nc = _NC_CACHE["nc"]
    in_maps = _host_prep(**inputs)
    res = run_bass_kernel_spmd(
        nc, in_maps, core_ids=list(range(N_CORES)), trace=trace, **kw)
    return _assemble(res.results), res


def kernel(**inputs):
    out, _ = run(inputs, trace=False)
    return out


# revision 24
# speedup vs baseline: 1.0956x; 1.0956x over previous
"""Multi-head self-attention (RoPE, causal) on 8 trn2 NeuronCores.

Sharding: core c -> batch b = c // 4, head group g = c % 4 (4 heads each).
Each core:
  - projects Q,K,V for its batch / its 4 heads (token-major), applies RoPE
    via a gathered cos/sin table, transposes Q,K to [dk, S] layout,
  - computes scores^T = K^T-chunk x Q (k on partitions), exp on ACT,
    then out^T[d, q] accumulated fully in PSUM via V'-stationary matmuls
    where V' carries 64 appended ones-columns so PSUM rows 64:128 hold the
    softmax sums pre-broadcast; normalization is then just an ACT
    partition-shift copy + fast reciprocal + one multiply per q-block,
  - exchanges the head-sharded attention output with its batch peers via
    an 8-way AllToAll per head pair (cross-batch blocks duplicated as
    padding so the program stays SPMD-uniform), gathers its q-slice rows
    with a per-core index table, and runs the output projection in two
    passes so the first AllToAll's data is consumed during attention.
Host only reshapes/transposes/casts inputs and concatenates outputs.
"""

import sys

for _p in ("/opt/trn_rl_repo",):
    if _p not in sys.path:
        sys.path.append(_p)

import numpy as np
import ml_dtypes

import concourse.bass as bass
import concourse.mybir as mybir
import concourse.tile as tile
from concourse import bacc
from concourse.bass import ds, ts
from concourse.bass_utils import run_bass_kernel_spmd
from concourse.masks import make_identity

BF16 = mybir.dt.bfloat16
F32 = mybir.dt.float32
I32 = mybir.dt.int32

B, S, D = 2, 2048, 1024
H, DK = 16, 64
THETA = 10000.0
MAXPOS = 2048
N_CORES = 8
GROUPS = 4          # head groups (cores) per batch
HPC = H // GROUPS   # heads per core = 4
QKV_COLS = 3 * HPC * DK        # 768 per-core projection width
QK_COLS = 2 * HPC * DK         # 512 (Q then K)
NSC = S // 128                 # 16 token chunks
NQC = S // 512                 # 4 q column-chunks
QSLICE = S // GROUPS           # 512 output tokens per core
MUL = mybir.AluOpType.mult
ADD = mybir.AluOpType.add
SUB = mybir.AluOpType.subtract
COPY_F = mybir.ActivationFunctionType.Copy
EXP_F = mybir.ActivationFunctionType.Exp


def _build():
    nc = bacc.Bacc("TRN2", num_devices=N_CORES)

    xT = nc.dram_tensor("xT", [D, S], BF16, kind="ExternalInput")
    wqkvT = nc.dram_tensor("wqkvT", [D, QKV_COLS], BF16, kind="ExternalInput")
    woT = nc.dram_tensor("woT", [D, D], BF16, kind="ExternalInput")
    cstab = nc.dram_tensor("cstab", [MAXPOS, 2 * DK], F32, kind="ExternalInput")
    pos = nc.dram_tensor("pos", [128, NSC], I32, kind="ExternalInput")
    tri = nc.dram_tensor("tri", [128, 128], BF16, kind="ExternalInput")
    wsel = nc.dram_tensor("wsel", [128, GROUPS], I32, kind="ExternalInput")
    finT = nc.dram_tensor("finT", [D, QSLICE], BF16, kind="ExternalOutput")

    with tile.TileContext(nc) as tc:
        with (
            tc.tile_pool(name="const", bufs=1) as constp,
            tc.tile_pool(name="wts", bufs=1) as wtsp,
            tc.tile_pool(name="seq", bufs=1) as seqp,
            tc.tile_pool(name="attp", bufs=1) as attp,
            tc.tile_pool(name="dram", bufs=1, space="DRAM") as dramp,
        ):
            # ---------------- early DMAs: only what phase 1 needs ----------
            wt = wtsp.tile([128, 8, QKV_COLS], BF16)       # [dchunk][768]
            for k in range(8):
                nc.sync.dma_start(out=wt[:, k, 0:512],
                                  in_=wqkvT[ts(k, 128), 0:512])

            pidx = constp.tile([128, NSC], I32)
            nc.sync.dma_start(out=pidx[:], in_=pos[:])
            tri_t = constp.tile([128, 128], BF16)
            nc.sync.dma_start(out=tri_t[:], in_=tri[:])
            wsel_sb = constp.tile([128, GROUPS], I32)
            nc.sync.dma_start(out=wsel_sb[:], in_=wsel[:])
            ident = constp.tile([128, 128], BF16)
            make_identity(nc, ident[:])

            # persistent per-core tensors
            qt = seqp.tile([128, 2, S], BF16)   # Q^T  [pair, dk(2x64), q]
            kt = seqp.tile([128, 2, S], BF16)   # K^T
            # V + 64 ones cols per head: [k-token][chunk][head][v64|ones64]
            vv = seqp.tile([128, NSC, HPC, 2 * DK], BF16)
            nc.vector.memset(vv[:, :, :, DK:], 1.0)
            attT = attp.tile([128, 2, S], BF16)
            rwo = attp.tile([128, 8, QSLICE], BF16)
            fe = attp.tile([128, 8, QSLICE], F32)
            wo = wtsp.tile([128, 8, D], BF16)   # loaded late (see below)

            # ------------- projection + RoPE + transposes (scoped x) -------
            with (
                tc.tile_pool(name="xap", bufs=1) as xap,
                tc.tile_pool(name="ropet", bufs=2) as ropet,
                tc.tile_pool(name="pbig", bufs=2, space="PSUM") as pbig,
                tc.tile_pool(name="psmall", bufs=2, space="PSUM") as psmall,
            ):
                xa = xap.tile([128, 8, S], BF16)           # resident x^T
                for k in range(8):
                    nc.sync.dma_start(
                        out=xa[:, k, 0:512], in_=xT[ts(k, 128), 0:512])
                for k in range(8):
                    nc.sync.dma_start(out=wt[:, k, 512:768],
                                      in_=wqkvT[ts(k, 128), 512:768])
                for q in range(1, 4):
                    for k in range(8):
                        nc.sync.dma_start(
                            out=xa[:, k, ts(q, 512)],
                            in_=xT[ts(k, 128), ts(q, 512)])
                cs = xap.tile([128, NSC, 2 * DK], F32)     # cos/sin gather
                for c in range(NSC):
                    nc.gpsimd.indirect_dma_start(
                        out=cs[:, c, :],
                        out_offset=None,
                        in_=cstab[:],
                        in_offset=bass.IndirectOffsetOnAxis(
                            ap=pidx[:, c:c + 1], axis=0),
                    )

                for sc in range(NSC):
                    ps = pbig.tile([128, QKV_COLS], F32, space="PSUM",
                                   tag="big")
                    for k in range(8):
                        nc.tensor.matmul(
                            ps[:, 0:512], lhsT=xa[:, k, ts(sc, 128)],
                            rhs=wt[:, k, 0:512],
                            start=(k == 0), stop=(k == 7),
                        )
                    for k in range(8):
                        nc.tensor.matmul(
                            ps[:, 512:768], lhsT=xa[:, k, ts(sc, 128)],
                            rhs=wt[:, k, 512:768],
                            start=(k == 0), stop=(k == 7),
                        )

                    # RoPE over the Q,K halves (cols 0:512), 8 blocks of 64
                    def qk_ap(off):
                        a = ps[:]
                        return bass.AP(a.tensor, a.offset + off,
                                       [a.ap[0], [DK, 2 * HPC], [2, DK // 2]])

                    def cs_ap(off):
                        a = cs[:, sc, :]
                        return bass.AP(a.tensor, a.offset + off,
                                       [a.ap[0], [0, 2 * HPC], [2, DK // 2]])

                    t1 = ropet.tile([128, 2 * HPC, DK // 2], F32, tag="t1")
                    t2 = ropet.tile([128, 2 * HPC, DK // 2], F32, tag="t2")
                    t3 = ropet.tile([128, 2 * HPC, DK // 2], F32, tag="t3")
                    t4 = ropet.tile([128, 2 * HPC, DK // 2], F32, tag="t4")
                    roped = ropet.tile([128, QK_COLS], BF16, tag="roped")

                    def roped_ap(off):
                        a = roped[:]
                        return bass.AP(a.tensor, a.offset + off,
                                       [a.ap[0], [DK, 2 * HPC], [2, DK // 2]])

                    nc.vector.tensor_tensor(t1[:], qk_ap(0), cs_ap(0), MUL)
                    nc.vector.tensor_tensor(t2[:], qk_ap(1), cs_ap(DK), MUL)
                    nc.vector.tensor_tensor(roped_ap(0), t1[:], t2[:], SUB)
                    nc.vector.tensor_tensor(t3[:], qk_ap(0), cs_ap(DK), MUL)
                    nc.vector.tensor_tensor(t4[:], qk_ap(1), cs_ap(0), MUL)
                    nc.vector.tensor_tensor(roped_ap(1), t3[:], t4[:], ADD)

                    # V columns (ones cols already set) -- on ACT
                    nc.scalar.activation(
                        vv[:, sc, :, 0:DK],
                        ps[:, 512:768].rearrange("p (h e) -> p h e", h=HPC),
                        COPY_F,
                    )

                    # transpose the 4 roped q/k 128-col blocks -> qt/kt
                    for t in range(4):
                        tp = psmall.tile([128, 128], BF16, space="PSUM",
                                         tag="small")
                        nc.tensor.transpose(tp[:], roped[:, ts(t, 128)],
                                            ident[:])
                        dst = qt if t < 2 else kt
                        nc.scalar.activation(dst[:, t % 2, ts(sc, 128)],
                                             tp[:], COPY_F)

                # late weight load: queued behind phase-1 DMAs on purpose
                for k in range(8):
                    nc.sync.dma_start(out=wo[:, k, :], in_=woT[ts(k, 128), :])

            # ------------- attention (per head) + 8-way A2A per head pair --
            agin = [dramp.tile([8 * 128, QSLICE], BF16, name=f"agin{p}")
                    for p in range(2)]
            agout = [dramp.tile([8 * 128, QSLICE], BF16, name=f"agout{p}")
                     for p in range(2)]

            with (
                tc.tile_pool(name="etp", bufs=3) as etp,
                tc.tile_pool(name="rsbp", bufs=2) as rsbp,
                tc.tile_pool(name="scoresp", bufs=2, space="PSUM") as scoresp,
                tc.tile_pool(name="vaccp", bufs=4, space="PSUM") as vaccp,
            ):
                def emit_sblock(h, j, et, qh):
                    # scores + one exp for k-chunk j, 1024-col window qh
                    pr, hf = h // 2, (h % 2) * DK
                    a0 = max(1024 * qh, 128 * j)
                    sp = scoresp.tile([128, 1024], F32, space="PSUM",
                                      tag="sc", name=f"sp{h}_{j}_{qh}")
                    for qq in (1024 * qh, 1024 * qh + 512):
                        a = max(a0, qq)
                        w = qq + 512 - a
                        if w <= 0:
                            continue
                        nc.tensor.matmul(
                            sp[:, ds(a - 1024 * qh, w)],
                            lhsT=kt[ds(hf, DK), pr, ts(j, 128)],
                            rhs=qt[ds(hf, DK), pr, ds(a, w)],
                            start=True, stop=True,
                        )
                    nc.scalar.activation(
                        et[:, ds(a0, 1024 * (qh + 1) - a0)],
                        sp[:, ds(a0 - 1024 * qh, 1024 * (qh + 1) - a0)],
                        EXP_F)
                    if qh == j // 8:   # diagonal block: mask q < k to 0
                        nc.vector.tensor_tensor(
                            et[:, ts(j, 128)], et[:, ts(j, 128)], tri_t[:],
                            MUL)

                def emit_vblock(h, j, et, vacc, qb):
                    a = max(512 * qb, 128 * j)
                    w = 512 * (qb + 1) - a
                    nc.tensor.matmul(
                        vacc[qb][:, ds(a - 512 * qb, w)],
                        lhsT=vv[:, j, h, :],
                        rhs=et[:, ds(a, w)],
                        start=(j == 0), stop=(j == 4 * qb + 3),
                    )

                def emit_normalize(h, vacc):
                    pr, hf = h // 2, (h % 2) * DK
                    for qc in range(NQC):
                        rsb = rsbp.tile([DK, 512], F32, tag="rsb")
                        nc.vector.tensor_copy(rsb[:], vacc[qc][ds(DK, DK), :])
                        nc.vector.reciprocal_approx_fast(rsb[:], rsb[:])
                        nc.vector.tensor_tensor(
                            attT[ds(hf, DK), pr, ts(qc, 512)],
                            vacc[qc][0:DK, :], rsb[:], MUL)
                        if h % 2 == 1:   # pair slice done -> stage for A2A
                            p = h // 2
                            for bh in range(2):
                                nc.sync.dma_start(
                                    out=agin[p][ds((4 * bh + qc) * 128, 128), :],
                                    in_=attT[:, p, ts(qc, 512)],
                                )

                def emit_a2a(p):
                    nc.gpsimd.collective_compute(
                        "AllToAll", mybir.AluOpType.bypass,
                        ins=[agin[p][:]], outs=[agout[p][:]],
                        replica_groups=[[0, 1, 2, 3, 4, 5, 6, 7]],
                    )
                    for j in range(GROUPS):
                        nc.gpsimd.indirect_dma_start(
                            out=rwo[:, 2 * j + p, :],
                            out_offset=None,
                            in_=agout[p][:],
                            in_offset=bass.IndirectOffsetOnAxis(
                                ap=wsel_sb[:, j:j + 1], axis=0),
                        )

                def emit_proj1(ec):
                    # shares the scores psum ring (same shape, half used)
                    fp = scoresp.tile([128, 1024], F32, space="PSUM",
                                      tag="sc", name=f"fp{ec}")
                    for i, dp in enumerate((0, 2, 4, 6)):
                        nc.tensor.matmul(
                            fp[:, 0:QSLICE], lhsT=wo[:, dp, ts(ec, 128)],
                            rhs=rwo[:, dp, :],
                            start=(i == 0), stop=(i == 3),
                        )
                    nc.vector.tensor_copy(fe[:, ec, :], fp[:, 0:QSLICE])

                for h in range(HPC):
                    vacc = [vaccp.tile([128, 512], F32, space="PSUM",
                                       tag="va", name=f"va{h}_{q}")
                            for q in range(NQC)]
                    ets = {}
                    for j in range(NSC):
                        et = etp.tile([128, S], BF16, tag="exp",
                                      name=f"et{h}_{j}")
                        ets[j] = et
                        # interleave: scores/exp for (j, qh) two chunks ahead
                        # of the V-matmuls for (j-2, qb) so ACT stays ahead
                        jv = j - 2
                        vqbs = (list(range(jv // 4, NQC)) if jv >= 0 else [])
                        for qh in range(j // 8, 2):
                            emit_sblock(h, j, et, qh)
                            while vqbs and (vqbs[0] <= 2 * qh + 1 or qh == 1):
                                emit_vblock(h, jv, ets[jv], vacc,
                                            vqbs.pop(0))
                        if h == 3 and j % 2 == 1:   # interleave proj pass 1
                            emit_proj1((j - 1) // 2)
                        ets.pop(j - 2, None)
                    for jv in (NSC - 2, NSC - 1):
                        for qb in range(jv // 4, NQC):
                            emit_vblock(h, jv, ets[jv], vacc, qb)
                    emit_normalize(h, vacc)
                    if h % 2 == 1:
                        emit_a2a(h // 2)

            # ------------- output projection pass 2 ------------------------
            with (
                tc.tile_pool(name="finp", bufs=2) as finp,
                tc.tile_pool(name="proj2p", bufs=2, space="PSUM") as proj2p,
            ):
                for ec in range(8):
                    fp = proj2p.tile([128, QSLICE], F32, space="PSUM",
                                     tag="p2")
                    for i, dp in enumerate((1, 3, 5, 7)):
                        nc.tensor.matmul(
                            fp[:], lhsT=wo[:, dp, ts(ec, 128)],
                            rhs=rwo[:, dp, :],
                            start=(i == 0), stop=(i == 3),
                        )
                    fin_sb = finp.tile([128, QSLICE], BF16, tag="fin")
                    nc.vector.tensor_tensor(fin_sb[:], fe[:, ec, :], fp[:],
                                            ADD)
                    nc.sync.dma_start(out=finT[ts(ec, 128), :], in_=fin_sb[:])

    nc.compile()
    return nc


def _host_prep(x, token_positions, W_qkv, W_o):
    bf16 = ml_dtypes.bfloat16
    xT = np.ascontiguousarray(np.transpose(x, (0, 2, 1))).astype(bf16)  # [B,D,S]

    # per-group W_qkv^T slices (Q rows pre-scaled by 1/sqrt(dk))
    wq = W_qkv[0 * D:1 * D] * np.float32(1.0 / np.sqrt(DK))
    wk = W_qkv[1 * D:2 * D]
    wv = W_qkv[2 * D:3 * D]
    wslices = []
    for g in range(GROUPS):
        rows = slice(g * HPC * DK, (g + 1) * HPC * DK)
        wsl = np.concatenate([wq[rows], wk[rows], wv[rows]], axis=0)  # [768, D]
        wslices.append(np.ascontiguousarray(wsl.T).astype(bf16))      # [D, 768]

    woT = np.ascontiguousarray(W_o.T).astype(bf16)                    # [D, D]

    idx = np.arange(DK // 2, dtype=np.float64)
    freqs = 1.0 / (THETA ** (2.0 * idx / DK))
    ang = np.arange(MAXPOS, dtype=np.float64)[:, None] * freqs[None, :]
    cstab = np.zeros((MAXPOS, 2 * DK), dtype=np.float32)
    cstab[:, 0:DK:2] = np.cos(ang)
    cstab[:, 1:DK:2] = np.cos(ang)
    cstab[:, DK::2] = np.sin(ang)
    cstab[:, DK + 1::2] = np.sin(ang)

    tri = (np.arange(128)[None, :] >= np.arange(128)[:, None]).astype(bf16)

    # pos transposed host-side: pos[r, c] = positions[128*c + r]
    posi = np.asarray(token_positions).astype(np.int32).reshape(B, NSC, 128)
    posi = np.ascontiguousarray(np.transpose(posi, (0, 2, 1)))  # [B,128,NSC]

    # wsel[r, j] = row of agout[h] holding peer (b, j)'s dims for my slice
    rr = np.arange(128)
    in_maps = []
    for c in range(N_CORES):
        b = c // GROUPS
        wsel = (512 * b + 128 * np.arange(GROUPS)[None, :]
                + rr[:, None]).astype(np.int32)
        in_maps.append({
            "xT": np.asarray(xT[b]),
            "wqkvT": wslices[c % GROUPS],
            "woT": woT,
            "cstab": cstab,
            "pos": posi[b],
            "tri": tri,
            "wsel": wsel,
        })
    return in_maps


def _assemble(results):
    out = np.empty((B, S, D), dtype=np.float32)
    for b in range(B):
        fullT = np.concatenate(
            [results[b * GROUPS + g]["finT"].astype(np.float32)
             for g in range(GROUPS)], axis=1)
        out[b] = fullT.T
    return out


_NC_CACHE = {}


def run(inputs, trace=False, **kw):
    if "nc" not in _NC_CACHE:
        _NC_CACHE["nc"] = _build()
    nc = _NC_CACHE["nc"]
    in_maps = _host_prep(**inputs)
    res = run_bass_kernel_spmd(
        nc, in_maps, core_ids=list(range(N_CORES)), trace=trace, **kw)
    return _assemble(res.results), res


def kernel(**inputs):
    out, _ = run(inputs, trace=False)
    return out


# revision 27
# speedup vs baseline: 1.2709x; 1.1600x over previous
"""Multi-head self-attention (RoPE, causal) on 8 trn2 NeuronCores.

Sharding: core c -> batch b = c // 4, head group g = c % 4 (4 heads each).
Each core:
  - projects Q,K,V for its batch / its 4 heads (token-major), applies RoPE
    via a gathered cos/sin table, transposes Q,K to [dk, S] layout,
  - computes scores^T = K^T-chunk x Q (k on partitions), exp on ACT,
    then out^T[d, q] accumulated fully in PSUM via V'-stationary matmuls
    where V' carries 64 appended ones-columns so PSUM rows 64:128 hold the
    softmax sums pre-broadcast; normalization is then just an ACT
    partition-shift copy + fast reciprocal + one multiply per q-block,
  - exchanges the head-sharded attention output with its batch peers via
    an 8-way AllToAll per head pair (cross-batch blocks duplicated as
    padding so the program stays SPMD-uniform), gathers its q-slice rows
    with a per-core index table, and runs the output projection in two
    passes so the first AllToAll's data is consumed during attention.
Host only reshapes/transposes/casts inputs and concatenates outputs.
"""

import sys

for _p in ("/opt/trn_rl_repo",):
    if _p not in sys.path:
        sys.path.append(_p)

import numpy as np
import ml_dtypes

import concourse.bass as bass
import concourse.mybir as mybir
import concourse.tile as tile
from concourse import bacc
from concourse.bass import ds, ts
from concourse.bass_utils import run_bass_kernel_spmd
from concourse.masks import make_identity

BF16 = mybir.dt.bfloat16
F32 = mybir.dt.float32
I32 = mybir.dt.int32

B, S, D = 2, 2048, 1024
H, DK = 16, 64
THETA = 10000.0
MAXPOS = 2048
N_CORES = 8
GROUPS = 4          # head groups (cores) per batch
HPC = H // GROUPS   # heads per core = 4
QKV_COLS = 3 * HPC * DK        # 768 per-core projection width
QK_COLS = 2 * HPC * DK         # 512 (Q then K)
NSC = S // 128                 # 16 token chunks
NQC = S // 512                 # 4 q column-chunks
QSLICE = S // GROUPS           # 512 output tokens per core
MUL = mybir.AluOpType.mult
ADD = mybir.AluOpType.add
SUB = mybir.AluOpType.subtract
COPY_F = mybir.ActivationFunctionType.Copy
EXP_F = mybir.ActivationFunctionType.Exp


def _build():
    nc = bacc.Bacc("TRN2", num_devices=N_CORES)

    xT = nc.dram_tensor("xT", [D, S], BF16, kind="ExternalInput")
    wqkvT = nc.dram_tensor("wqkvT", [D, QKV_COLS], BF16, kind="ExternalInput")
    woT = nc.dram_tensor("woT", [D, D], BF16, kind="ExternalInput")
    cstab = nc.dram_tensor("cstab", [MAXPOS, 2 * DK], F32, kind="ExternalInput")
    pos = nc.dram_tensor("pos", [128, NSC], I32, kind="ExternalInput")
    tri = nc.dram_tensor("tri", [128, 128], BF16, kind="ExternalInput")
    wsel = nc.dram_tensor("wsel", [128, GROUPS], I32, kind="ExternalInput")
    finT = nc.dram_tensor("finT", [D, QSLICE], BF16, kind="ExternalOutput")

    with tile.TileContext(nc) as tc:
        with (
            tc.tile_pool(name="const", bufs=1) as constp,
            tc.tile_pool(name="wts", bufs=1) as wtsp,
            tc.tile_pool(name="seq", bufs=1) as seqp,
            tc.tile_pool(name="attp", bufs=1) as attp,
            tc.tile_pool(name="dram", bufs=1, space="DRAM") as dramp,
        ):
            # ---------------- early DMAs: only what phase 1 needs ----------
            wt = wtsp.tile([128, 8, QKV_COLS], BF16)       # [dchunk][768]
            for k in range(8):
                nc.sync.dma_start(out=wt[:, k, 0:512],
                                  in_=wqkvT[ts(k, 128), 0:512])

            pidx = constp.tile([128, NSC], I32)
            nc.sync.dma_start(out=pidx[:], in_=pos[:])
            tri_t = constp.tile([128, 128], BF16)
            nc.sync.dma_start(out=tri_t[:], in_=tri[:])
            wsel_sb = constp.tile([128, GROUPS], I32)
            nc.sync.dma_start(out=wsel_sb[:], in_=wsel[:])
            ident = constp.tile([128, 128], BF16)
            make_identity(nc, ident[:])

            # persistent per-core tensors
            # Q^T zero-padded per head to full 128 contraction rows: head h
            # occupies rows (h%2)*64..+64, the other 64 rows stay zero so
            # scores matmuls contract over the full kt pair tile
            qt = seqp.tile([128, HPC, S], BF16)
            nc.vector.memset(qt[:], 0.0)
            kt = seqp.tile([128, 2, S], BF16)   # K^T  [pair, dk(2x64), k]
            # V + 64 ones cols per head: [k-token][chunk][head][v64|ones64]
            vv = seqp.tile([128, NSC, HPC, 2 * DK], BF16)
            nc.vector.memset(vv[:, :, :, DK:], 1.0)
            attT = attp.tile([128, 2, S], BF16)
            rwo = attp.tile([128, 8, QSLICE], BF16)
            fe = attp.tile([128, 8, QSLICE], F32)
            wo = wtsp.tile([128, 8, D], BF16)   # loaded late (see below)

            # ------------- projection + RoPE + transposes (scoped x) -------
            with (
                tc.tile_pool(name="xap", bufs=1) as xap,
                tc.tile_pool(name="ropet", bufs=2) as ropet,
                tc.tile_pool(name="pbig", bufs=2, space="PSUM") as pbig,
                tc.tile_pool(name="psmall", bufs=2, space="PSUM") as psmall,
            ):
                xa = xap.tile([128, 8, S], BF16)           # resident x^T
                for k in range(8):
                    nc.sync.dma_start(
                        out=xa[:, k, 0:512], in_=xT[ts(k, 128), 0:512])
                for k in range(8):
                    nc.sync.dma_start(out=wt[:, k, 512:768],
                                      in_=wqkvT[ts(k, 128), 512:768])
                for q in range(1, 4):
                    for k in range(8):
                        nc.sync.dma_start(
                            out=xa[:, k, ts(q, 512)],
                            in_=xT[ts(k, 128), ts(q, 512)])
                cs = xap.tile([128, NSC, 2 * DK], F32)     # cos/sin gather
                for c in range(NSC):
                    nc.gpsimd.indirect_dma_start(
                        out=cs[:, c, :],
                        out_offset=None,
                        in_=cstab[:],
                        in_offset=bass.IndirectOffsetOnAxis(
                            ap=pidx[:, c:c + 1], axis=0),
                    )

                for sc in range(NSC):
                    ps = pbig.tile([128, QKV_COLS], F32, space="PSUM",
                                   tag="big")
                    for k in range(8):
                        nc.tensor.matmul(
                            ps[:, 0:512], lhsT=xa[:, k, ts(sc, 128)],
                            rhs=wt[:, k, 0:512],
                            start=(k == 0), stop=(k == 7),
                        )
                    for k in range(8):
                        nc.tensor.matmul(
                            ps[:, 512:768], lhsT=xa[:, k, ts(sc, 128)],
                            rhs=wt[:, k, 512:768],
                            start=(k == 0), stop=(k == 7),
                        )

                    # RoPE over the Q,K halves (cols 0:512), 8 blocks of 64
                    def qk_ap(off):
                        a = ps[:]
                        return bass.AP(a.tensor, a.offset + off,
                                       [a.ap[0], [DK, 2 * HPC], [2, DK // 2]])

                    def cs_ap(off):
                        a = cs[:, sc, :]
                        return bass.AP(a.tensor, a.offset + off,
                                       [a.ap[0], [0, 2 * HPC], [2, DK // 2]])

                    t1 = ropet.tile([128, 2 * HPC, DK // 2], F32, tag="t1")
                    t2 = ropet.tile([128, 2 * HPC, DK // 2], F32, tag="t2")
                    t3 = ropet.tile([128, 2 * HPC, DK // 2], F32, tag="t3")
                    t4 = ropet.tile([128, 2 * HPC, DK // 2], F32, tag="t4")
                    roped = ropet.tile([128, QK_COLS], BF16, tag="roped")

                    def roped_ap(off):
                        a = roped[:]
                        return bass.AP(a.tensor, a.offset + off,
                                       [a.ap[0], [DK, 2 * HPC], [2, DK // 2]])

                    nc.vector.tensor_tensor(t1[:], qk_ap(0), cs_ap(0), MUL)
                    nc.vector.tensor_tensor(t2[:], qk_ap(1), cs_ap(DK), MUL)
                    nc.vector.tensor_tensor(roped_ap(0), t1[:], t2[:], SUB)
                    nc.vector.tensor_tensor(t3[:], qk_ap(0), cs_ap(DK), MUL)
                    nc.vector.tensor_tensor(t4[:], qk_ap(1), cs_ap(0), MUL)
                    nc.vector.tensor_tensor(roped_ap(1), t3[:], t4[:], ADD)

                    # V columns (ones cols already set) -- on ACT
                    nc.scalar.activation(
                        vv[:, sc, :, 0:DK],
                        ps[:, 512:768].rearrange("p (h e) -> p h e", h=HPC),
                        COPY_F,
                    )

                    # transpose the 4 roped q/k 128-col blocks -> qt/kt
                    for t in range(4):
                        tp = psmall.tile([128, 128], BF16, space="PSUM",
                                         tag="small")
                        nc.tensor.transpose(tp[:], roped[:, ts(t, 128)],
                                            ident[:])
                        if t < 2:   # q pair t -> padded per-head slots
                            nc.scalar.activation(
                                qt[0:DK, 2 * t, ts(sc, 128)], tp[0:DK, :],
                                COPY_F)
                            nc.scalar.activation(
                                qt[ds(DK, DK), 2 * t + 1, ts(sc, 128)],
                                tp[ds(DK, DK), :], COPY_F)
                        else:
                            nc.scalar.activation(kt[:, t % 2, ts(sc, 128)],
                                                 tp[:], COPY_F)

                # late weight load: queued behind phase-1 DMAs on purpose
                for k in range(8):
                    nc.sync.dma_start(out=wo[:, k, :], in_=woT[ts(k, 128), :])

            # ------------- attention (per head) + 8-way A2A per head pair --
            agin = [dramp.tile([8 * 128, QSLICE], BF16, name=f"agin{p}")
                    for p in range(2)]
            agout = [dramp.tile([8 * 128, QSLICE], BF16, name=f"agout{p}")
                     for p in range(2)]

            with (
                tc.tile_pool(name="etp", bufs=3) as etp,
                tc.tile_pool(name="rsbp", bufs=2) as rsbp,
                tc.tile_pool(name="scoresp", bufs=2, space="PSUM") as scoresp,
                tc.tile_pool(name="vaccp", bufs=4, space="PSUM") as vaccp,
            ):
                def emit_sblock(h, j, et, qh):
                    # scores + one exp for k-chunk j, 1024-col window qh
                    pr, hf = h // 2, (h % 2) * DK
                    a0 = max(1024 * qh, 128 * j)
                    sp = scoresp.tile([128, 1024], F32, space="PSUM",
                                      tag="sc", name=f"sp{h}_{j}_{qh}")
                    for qq in (1024 * qh, 1024 * qh + 512):
                        a = max(a0, qq)
                        w = qq + 512 - a
                        if w <= 0:
                            continue
                        nc.tensor.matmul(
                            sp[:, ds(a - 1024 * qh, w)],
                            lhsT=kt[:, pr, ts(j, 128)],
                            rhs=qt[:, h, ds(a, w)],
                            start=True, stop=True,
                        )
                    nc.scalar.activation(
                        et[:, ds(a0, 1024 * (qh + 1) - a0)],
                        sp[:, ds(a0 - 1024 * qh, 1024 * (qh + 1) - a0)],
                        EXP_F)
                    if qh == j // 8:   # diagonal block: mask q < k to 0
                        nc.vector.tensor_tensor(
                            et[:, ts(j, 128)], et[:, ts(j, 128)], tri_t[:],
                            MUL)

                def emit_vblock(h, j, et, vacc, qb):
                    a = max(512 * qb, 128 * j)
                    w = 512 * (qb + 1) - a
                    nc.tensor.matmul(
                        vacc[qb][:, ds(a - 512 * qb, w)],
                        lhsT=vv[:, j, h, :],
                        rhs=et[:, ds(a, w)],
                        start=(j == 0), stop=(j == 4 * qb + 3),
                    )

                def emit_normalize(h, vacc):
                    pr, hf = h // 2, (h % 2) * DK
                    for qc in range(NQC):
                        rsb = rsbp.tile([DK, 512], F32, tag="rsb")
                        nc.vector.tensor_copy(rsb[:], vacc[qc][ds(DK, DK), :])
                        nc.vector.reciprocal_approx_fast(rsb[:], rsb[:])
                        nc.vector.tensor_tensor(
                            attT[ds(hf, DK), pr, ts(qc, 512)],
                            vacc[qc][0:DK, :], rsb[:], MUL)
                        if h % 2 == 1:   # pair slice done -> stage for A2A
                            p = h // 2
                            for bh in range(2):
                                nc.sync.dma_start(
                                    out=agin[p][ds((4 * bh + qc) * 128, 128), :],
                                    in_=attT[:, p, ts(qc, 512)],
                                )

                def emit_a2a(p):
                    nc.gpsimd.collective_compute(
                        "AllToAll", mybir.AluOpType.bypass,
                        ins=[agin[p][:]], outs=[agout[p][:]],
                        replica_groups=[[0, 1, 2, 3, 4, 5, 6, 7]],
                    )
                    for j in range(GROUPS):
                        nc.gpsimd.indirect_dma_start(
                            out=rwo[:, 2 * j + p, :],
                            out_offset=None,
                            in_=agout[p][:],
                            in_offset=bass.IndirectOffsetOnAxis(
                                ap=wsel_sb[:, j:j + 1], axis=0),
                        )

                def emit_proj1(ec):
                    # shares the scores psum ring (same shape, half used)
                    fp = scoresp.tile([128, 1024], F32, space="PSUM",
                                      tag="sc", name=f"fp{ec}")
                    for i, dp in enumerate((0, 2, 4, 6)):
                        nc.tensor.matmul(
                            fp[:, 0:QSLICE], lhsT=wo[:, dp, ts(ec, 128)],
                            rhs=rwo[:, dp, :],
                            start=(i == 0), stop=(i == 3),
                        )
                    nc.vector.tensor_copy(fe[:, ec, :], fp[:, 0:QSLICE])

                for h in range(HPC):
                    vacc = [vaccp.tile([128, 512], F32, space="PSUM",
                                       tag="va", name=f"va{h}_{q}")
                            for q in range(NQC)]
                    ets = {}
                    for j in range(NSC):
                        et = etp.tile([128, S], BF16, tag="exp",
                                      name=f"et{h}_{j}")
                        ets[j] = et
                        # interleave: scores/exp for (j, qh) two chunks ahead
                        # of the V-matmuls for (j-2, qb) so ACT stays ahead
                        jv = j - 2
                        vqbs = (list(range(jv // 4, NQC)) if jv >= 0 else [])
                        for qh in range(j // 8, 2):
                            emit_sblock(h, j, et, qh)
                            while vqbs and (vqbs[0] <= 2 * qh + 1 or qh == 1):
                                emit_vblock(h, jv, ets[jv], vacc,
                                            vqbs.pop(0))
                        if h == 3 and j % 2 == 1:   # interleave proj pass 1
                            emit_proj1((j - 1) // 2)
                        ets.pop(j - 2, None)
                    for jv in (NSC - 2, NSC - 1):
                        for qb in range(jv // 4, NQC):
                            emit_vblock(h, jv, ets[jv], vacc, qb)
                    emit_normalize(h, vacc)
                    if h % 2 == 1:
                        emit_a2a(h // 2)

            # ------------- output projection pass 2 ------------------------
            with (
                tc.tile_pool(name="finp", bufs=2) as finp,
                tc.tile_pool(name="proj2p", bufs=2, space="PSUM") as proj2p,
            ):
                for ec in range(8):
                    fp = proj2p.tile([128, QSLICE], F32, space="PSUM",
                                     tag="p2")
                    for i, dp in enumerate((1, 3, 5, 7)):
                        nc.tensor.matmul(
                            fp[:], lhsT=wo[:, dp, ts(ec, 128)],
                            rhs=rwo[:, dp, :],
                            start=(i == 0), stop=(i == 3),
                        )
                    fin_sb = finp.tile([128, QSLICE], BF16, tag="fin")
                    nc.vector.tensor_tensor(fin_sb[:], fe[:, ec, :], fp[:],
                                            ADD)
                    nc.sync.dma_start(out=finT[ts(ec, 128), :], in_=fin_sb[:])

    nc.compile()
    return nc


def _host_prep(x, token_positions, W_qkv, W_o):
    bf16 = ml_dtypes.bfloat16
    xT = np.ascontiguousarray(np.transpose(x, (0, 2, 1))).astype(bf16)  # [B,D,S]

    # per-group W_qkv^T slices (Q rows pre-scaled by 1/sqrt(dk))
    wq = W_qkv[0 * D:1 * D] * np.float32(1.0 / np.sqrt(DK))
    wk = W_qkv[1 * D:2 * D]
    wv = W_qkv[2 * D:3 * D]
    wslices = []
    for g in range(GROUPS):
        rows = slice(g * HPC * DK, (g + 1) * HPC * DK)
        wsl = np.concatenate([wq[rows], wk[rows], wv[rows]], axis=0)  # [768, D]
        wslices.append(np.ascontiguousarray(wsl.T).astype(bf16))      # [D, 768]

    woT = np.ascontiguousarray(W_o.T).astype(bf16)                    # [D, D]

    idx = np.arange(DK // 2, dtype=np.float64)
    freqs = 1.0 / (THETA ** (2.0 * idx / DK))
    ang = np.arange(MAXPOS, dtype=np.float64)[:, None] * freqs[None, :]
    cstab = np.zeros((MAXPOS, 2 * DK), dtype=np.float32)
    cstab[:, 0:DK:2] = np.cos(ang)
    cstab[:, 1:DK:2] = np.cos(ang)
    cstab[:, DK::2] = np.sin(ang)
    cstab[:, DK + 1::2] = np.sin(ang)

    tri = (np.arange(128)[None, :] >= np.arange(128)[:, None]).astype(bf16)

    # pos transposed host-side: pos[r, c] = positions[128*c + r]
    posi = np.asarray(token_positions).astype(np.int32).reshape(B, NSC, 128)
    posi = np.ascontiguousarray(np.transpose(posi, (0, 2, 1)))  # [B,128,NSC]

    # wsel[r, j] = row of agout[h] holding peer (b, j)'s dims for my slice
    rr = np.arange(128)
    in_maps = []
    for c in range(N_CORES):
        b = c // GROUPS
        wsel = (512 * b + 128 * np.arange(GROUPS)[None, :]
                + rr[:, None]).astype(np.int32)
        in_maps.append({
            "xT": np.asarray(xT[b]),
            "wqkvT": wslices[c % GROUPS],
            "woT": woT,
            "cstab": cstab,
            "pos": posi[b],
            "tri": tri,
            "wsel": wsel,
        })
    return in_maps


def _assemble(results):
    out = np.empty((B, S, D), dtype=np.float32)
    for b in range(B):
        fullT = np.concatenate(
            [results[b * GROUPS + g]["finT"].astype(np.float32)
             for g in range(GROUPS)], axis=1)
        out[b] = fullT.T
    return out


_NC_CACHE = {}


def run(inputs, trace=False, **kw):
    if "nc" not in _NC_CACHE:
        _NC_CACHE["nc"] = _build()
    nc = _NC_CACHE["nc"]
    in_maps = _host_prep(**inputs)
    res = run_bass_kernel_spmd(
        nc, in_maps, core_ids=list(range(N_CORES)), trace=trace, **kw)
    return _assemble(res.results), res


def kernel(**inputs):
    out, _ = run(inputs, trace=False)
    return out


# revision 30
# speedup vs baseline: 1.3899x; 1.0937x over previous
"""Multi-head self-attention (RoPE, causal) on 8 trn2 NeuronCores.

Sharding: core c -> batch b = c // 4, head group g = c % 4 (4 heads each).
Each core:
  - projects Q,K,V for its batch / its 4 heads (token-major), applies RoPE
    via a gathered cos/sin table, transposes Q,K to [dk, S] layout,
  - computes scores^T = K^T-chunk x Q (k on partitions), exp on ACT,
    then out^T[d, q] accumulated fully in PSUM via V'-stationary matmuls
    where V' carries 64 appended ones-columns so PSUM rows 64:128 hold the
    softmax sums pre-broadcast; normalization is then just an ACT
    partition-shift copy + fast reciprocal + one multiply per q-block,
  - exchanges the head-sharded attention output with its batch peers via
    an 8-way AllToAll per head pair (cross-batch blocks duplicated as
    padding so the program stays SPMD-uniform), gathers its q-slice rows
    with a per-core index table, and runs the output projection in two
    passes so the first AllToAll's data is consumed during attention.
Host only reshapes/transposes/casts inputs and concatenates outputs.
"""

import sys

for _p in ("/opt/trn_rl_repo",):
    if _p not in sys.path:
        sys.path.append(_p)

import numpy as np
import ml_dtypes

import concourse.bass as bass
import concourse.mybir as mybir
import concourse.tile as tile
from concourse import bacc
from concourse.bass import ds, ts
from concourse.bass_utils import run_bass_kernel_spmd
from concourse.masks import make_identity

BF16 = mybir.dt.bfloat16
F32 = mybir.dt.float32
I32 = mybir.dt.int32

B, S, D = 2, 2048, 1024
H, DK = 16, 64
THETA = 10000.0
MAXPOS = 2048
N_CORES = 8
GROUPS = 4          # head groups (cores) per batch
HPC = H // GROUPS   # heads per core = 4
QKV_COLS = 3 * HPC * DK        # 768 per-core projection width
QK_COLS = 2 * HPC * DK         # 512 (Q then K)
NSC = S // 128                 # 16 token chunks
NQC = S // 512                 # 4 q column-chunks
QSLICE = S // GROUPS           # 512 output tokens per core
MUL = mybir.AluOpType.mult
ADD = mybir.AluOpType.add
SUB = mybir.AluOpType.subtract
COPY_F = mybir.ActivationFunctionType.Copy
EXP_F = mybir.ActivationFunctionType.Exp


def _build():
    nc = bacc.Bacc("TRN2", num_devices=N_CORES)

    xT = nc.dram_tensor("xT", [D, S], BF16, kind="ExternalInput")
    wqkvT = nc.dram_tensor("wqkvT", [D, QKV_COLS], BF16, kind="ExternalInput")
    woT = nc.dram_tensor("woT", [D, D], BF16, kind="ExternalInput")
    cstab = nc.dram_tensor("cstab", [MAXPOS, 2 * DK], F32, kind="ExternalInput")
    pos = nc.dram_tensor("pos", [128, NSC], I32, kind="ExternalInput")
    tri = nc.dram_tensor("tri", [128, 128], BF16, kind="ExternalInput")
    wsel = nc.dram_tensor("wsel", [128, GROUPS], I32, kind="ExternalInput")
    finT = nc.dram_tensor("finT", [D, QSLICE], BF16, kind="ExternalOutput")

    with tile.TileContext(nc) as tc:
        with (
            tc.tile_pool(name="const", bufs=1) as constp,
            tc.tile_pool(name="wts", bufs=1) as wtsp,
            tc.tile_pool(name="seq", bufs=1) as seqp,
            tc.tile_pool(name="attp", bufs=1) as attp,
            tc.tile_pool(name="dram", bufs=1, space="DRAM") as dramp,
        ):
            # ---------------- early DMAs: only what phase 1 needs ----------
            wt = wtsp.tile([128, 8, QKV_COLS], BF16)       # [dchunk][768]
            for k in range(8):
                nc.sync.dma_start(out=wt[:, k, 0:512],
                                  in_=wqkvT[ts(k, 128), 0:512])

            pidx = constp.tile([128, NSC], I32)
            nc.sync.dma_start(out=pidx[:], in_=pos[:])
            tri_t = constp.tile([128, 128], BF16)
            nc.sync.dma_start(out=tri_t[:], in_=tri[:])
            wsel_sb = constp.tile([128, GROUPS], I32)
            nc.sync.dma_start(out=wsel_sb[:], in_=wsel[:])
            ident = constp.tile([128, 128], BF16)
            make_identity(nc, ident[:])

            # persistent per-core tensors
            # Q^T zero-padded per head to full 128 contraction rows: head h
            # occupies rows (h%2)*64..+64, the other 64 rows stay zero so
            # scores matmuls contract over the full kt pair tile
            qt = seqp.tile([128, HPC, S], BF16)
            nc.vector.memset(qt[:], 0.0)
            kt = seqp.tile([128, 2, S], BF16)   # K^T  [pair, dk(2x64), k]
            # V + 64 ones cols per head: [k-token][chunk][head][v64|ones64]
            vv = seqp.tile([128, NSC, HPC, 2 * DK], BF16)
            nc.vector.memset(vv[:, :, :, DK:], 1.0)
            attT = attp.tile([128, 2, S], BF16)
            rwo = attp.tile([128, 8, QSLICE], BF16)
            fe = attp.tile([128, 8, QSLICE], F32)
            wo = wtsp.tile([128, 8, D], BF16)   # loaded late (see below)

            # ------------- projection + RoPE + transposes (scoped x) -------
            with (
                tc.tile_pool(name="xap", bufs=1) as xap,
                tc.tile_pool(name="ropet", bufs=2) as ropet,
                tc.tile_pool(name="pbig", bufs=2, space="PSUM") as pbig,
                tc.tile_pool(name="psmall", bufs=2, space="PSUM") as psmall,
            ):
                xa = xap.tile([128, 8, S], BF16)           # resident x^T
                for k in range(8):
                    nc.sync.dma_start(
                        out=xa[:, k, 0:512], in_=xT[ts(k, 128), 0:512])
                for k in range(8):
                    nc.sync.dma_start(out=wt[:, k, 512:768],
                                      in_=wqkvT[ts(k, 128), 512:768])
                for q in range(1, 4):
                    for k in range(8):
                        nc.sync.dma_start(
                            out=xa[:, k, ts(q, 512)],
                            in_=xT[ts(k, 128), ts(q, 512)])
                cs = xap.tile([128, NSC, 2 * DK], F32)     # cos/sin gather
                for c in range(NSC):
                    nc.gpsimd.indirect_dma_start(
                        out=cs[:, c, :],
                        out_offset=None,
                        in_=cstab[:],
                        in_offset=bass.IndirectOffsetOnAxis(
                            ap=pidx[:, c:c + 1], axis=0),
                    )
                # tiny dummy collective: absorbs the inter-core arrival skew
                # early (fully overlapped by phase 1) so the first real A2A
                # sees a short entry wait
                dumin = dramp.tile([8, 32], BF16, name="dumin")
                dumout = dramp.tile([8, 32], BF16, name="dumout")
                nc.gpsimd.collective_compute(
                    "AllToAll", mybir.AluOpType.bypass,
                    ins=[dumin[:]], outs=[dumout[:]],
                    replica_groups=[[0, 1, 2, 3, 4, 5, 6, 7]],
                )

                for sc in range(NSC):
                    ps = pbig.tile([128, QKV_COLS], F32, space="PSUM",
                                   tag="big")
                    for k in range(8):
                        nc.tensor.matmul(
                            ps[:, 0:512], lhsT=xa[:, k, ts(sc, 128)],
                            rhs=wt[:, k, 0:512],
                            start=(k == 0), stop=(k == 7),
                        )
                    for k in range(8):
                        nc.tensor.matmul(
                            ps[:, 512:768], lhsT=xa[:, k, ts(sc, 128)],
                            rhs=wt[:, k, 512:768],
                            start=(k == 0), stop=(k == 7),
                        )

                    # RoPE over the Q,K halves (cols 0:512), 8 blocks of 64
                    def qk_ap(off):
                        a = ps[:]
                        return bass.AP(a.tensor, a.offset + off,
                                       [a.ap[0], [DK, 2 * HPC], [2, DK // 2]])

                    def cs_ap(off):
                        a = cs[:, sc, :]
                        return bass.AP(a.tensor, a.offset + off,
                                       [a.ap[0], [0, 2 * HPC], [2, DK // 2]])

                    t1 = ropet.tile([128, 2 * HPC, DK // 2], F32, tag="t1")
                    t2 = ropet.tile([128, 2 * HPC, DK // 2], F32, tag="t2")
                    t3 = ropet.tile([128, 2 * HPC, DK // 2], F32, tag="t3")
                    t4 = ropet.tile([128, 2 * HPC, DK // 2], F32, tag="t4")
                    roped = ropet.tile([128, QK_COLS], BF16, tag="roped")

                    def roped_ap(off):
                        a = roped[:]
                        return bass.AP(a.tensor, a.offset + off,
                                       [a.ap[0], [DK, 2 * HPC], [2, DK // 2]])

                    nc.vector.tensor_tensor(t1[:], qk_ap(0), cs_ap(0), MUL)
                    nc.vector.tensor_tensor(t2[:], qk_ap(1), cs_ap(DK), MUL)
                    nc.vector.tensor_tensor(roped_ap(0), t1[:], t2[:], SUB)
                    nc.vector.tensor_tensor(t3[:], qk_ap(0), cs_ap(DK), MUL)
                    nc.vector.tensor_tensor(t4[:], qk_ap(1), cs_ap(0), MUL)
                    nc.vector.tensor_tensor(roped_ap(1), t3[:], t4[:], ADD)

                    # V columns (ones cols already set) -- on ACT
                    nc.scalar.activation(
                        vv[:, sc, :, 0:DK],
                        ps[:, 512:768].rearrange("p (h e) -> p h e", h=HPC),
                        COPY_F,
                    )

                    # transpose the 4 roped q/k 128-col blocks -> qt/kt
                    for t in range(4):
                        tp = psmall.tile([128, 128], BF16, space="PSUM",
                                         tag="small")
                        nc.tensor.transpose(tp[:], roped[:, ts(t, 128)],
                                            ident[:])
                        if t < 2:   # q pair t -> padded per-head slots
                            nc.scalar.activation(
                                qt[0:DK, 2 * t, ts(sc, 128)], tp[0:DK, :],
                                COPY_F)
                            nc.scalar.activation(
                                qt[ds(DK, DK), 2 * t + 1, ts(sc, 128)],
                                tp[ds(DK, DK), :], COPY_F)
                        else:
                            nc.scalar.activation(kt[:, t % 2, ts(sc, 128)],
                                                 tp[:], COPY_F)

                # late weight load: queued behind phase-1 DMAs on purpose
                for k in range(8):
                    nc.sync.dma_start(out=wo[:, k, :], in_=woT[ts(k, 128), :])

            # ------------- attention (per head) + 8-way A2A per head pair --
            agin = [dramp.tile([8 * 128, QSLICE], BF16, name=f"agin{p}")
                    for p in range(2)]
            agout = [dramp.tile([8 * 128, QSLICE], BF16, name=f"agout{p}")
                     for p in range(2)]

            with (
                tc.tile_pool(name="etp", bufs=3) as etp,
                tc.tile_pool(name="rsbp", bufs=2) as rsbp,
                tc.tile_pool(name="scoresp", bufs=2, space="PSUM") as scoresp,
                tc.tile_pool(name="vaccp", bufs=4, space="PSUM") as vaccp,
            ):
                def emit_sblock(h, j, et, qh):
                    # scores + one exp for k-chunk j, 1024-col window qh
                    pr, hf = h // 2, (h % 2) * DK
                    a0 = max(1024 * qh, 128 * j)
                    sp = scoresp.tile([128, 1024], F32, space="PSUM",
                                      tag="sc", name=f"sp{h}_{j}_{qh}")
                    for qq in (1024 * qh, 1024 * qh + 512):
                        a = max(a0, qq)
                        w = qq + 512 - a
                        if w <= 0:
                            continue
                        nc.tensor.matmul(
                            sp[:, ds(a - 1024 * qh, w)],
                            lhsT=kt[:, pr, ts(j, 128)],
                            rhs=qt[:, h, ds(a, w)],
                            start=True, stop=True,
                        )
                    nc.scalar.activation(
                        et[:, ds(a0, 1024 * (qh + 1) - a0)],
                        sp[:, ds(a0 - 1024 * qh, 1024 * (qh + 1) - a0)],
                        EXP_F)
                    if qh == j // 8:   # diagonal block: mask q < k to 0
                        nc.vector.tensor_tensor(
                            et[:, ts(j, 128)], et[:, ts(j, 128)], tri_t[:],
                            MUL)

                def emit_vblock(h, j, et, vacc, qb):
                    a = max(512 * qb, 128 * j)
                    w = 512 * (qb + 1) - a
                    nc.tensor.matmul(
                        vacc[qb][:, ds(a - 512 * qb, w)],
                        lhsT=vv[:, j, h, :],
                        rhs=et[:, ds(a, w)],
                        start=(j == 0), stop=(j == 4 * qb + 3),
                    )

                def emit_normalize(h, vacc):
                    pr, hf = h // 2, (h % 2) * DK
                    for qc in range(NQC):
                        rsb = rsbp.tile([DK, 512], F32, tag="rsb")
                        nc.vector.tensor_copy(rsb[:], vacc[qc][ds(DK, DK), :])
                        nc.vector.reciprocal_approx_fast(rsb[:], rsb[:])
                        nc.vector.tensor_tensor(
                            attT[ds(hf, DK), pr, ts(qc, 512)],
                            vacc[qc][0:DK, :], rsb[:], MUL)
                        if h % 2 == 1:   # pair slice done -> stage for A2A
                            p = h // 2
                            for bh in range(2):
                                nc.sync.dma_start(
                                    out=agin[p][ds((4 * bh + qc) * 128, 128), :],
                                    in_=attT[:, p, ts(qc, 512)],
                                )

                def emit_a2a(p):
                    nc.gpsimd.collective_compute(
                        "AllToAll", mybir.AluOpType.bypass,
                        ins=[agin[p][:]], outs=[agout[p][:]],
                        replica_groups=[[0, 1, 2, 3, 4, 5, 6, 7]],
                    )
                    for j in range(GROUPS):
                        nc.gpsimd.indirect_dma_start(
                            out=rwo[:, 2 * j + p, :],
                            out_offset=None,
                            in_=agout[p][:],
                            in_offset=bass.IndirectOffsetOnAxis(
                                ap=wsel_sb[:, j:j + 1], axis=0),
                        )

                def emit_proj1(ec):
                    # shares the scores psum ring (same shape, half used)
                    fp = scoresp.tile([128, 1024], F32, space="PSUM",
                                      tag="sc", name=f"fp{ec}")
                    for i, dp in enumerate((0, 2, 4, 6)):
                        nc.tensor.matmul(
                            fp[:, 0:QSLICE], lhsT=wo[:, dp, ts(ec, 128)],
                            rhs=rwo[:, dp, :],
                            start=(i == 0), stop=(i == 3),
                        )
                    nc.vector.tensor_copy(fe[:, ec, :], fp[:, 0:QSLICE])

                for h in range(HPC):
                    vacc = [vaccp.tile([128, 512], F32, space="PSUM",
                                       tag="va", name=f"va{h}_{q}")
                            for q in range(NQC)]
                    ets = {}
                    for j in range(NSC):
                        et = etp.tile([128, S], BF16, tag="exp",
                                      name=f"et{h}_{j}")
                        ets[j] = et
                        # interleave: scores/exp for (j, qh) two chunks ahead
                        # of the V-matmuls for (j-2, qb) so ACT stays ahead
                        jv = j - 2
                        vqbs = (list(range(jv // 4, NQC)) if jv >= 0 else [])
                        for qh in range(j // 8, 2):
                            emit_sblock(h, j, et, qh)
                            while vqbs and (vqbs[0] <= 2 * qh + 1 or qh == 1):
                                emit_vblock(h, jv, ets[jv], vacc,
                                            vqbs.pop(0))
                        ets.pop(j - 2, None)
                    for jv in (NSC - 2, NSC - 1):
                        for qb in range(jv // 4, NQC):
                            emit_vblock(h, jv, ets[jv], vacc, qb)
                    emit_normalize(h, vacc)
                    if h % 2 == 1:
                        emit_a2a(h // 2)
                # proj pass 1 runs during the second A2A's flight
                for ec in range(8):
                    emit_proj1(ec)

            # ------------- output projection pass 2 ------------------------
            with (
                tc.tile_pool(name="finp", bufs=2) as finp,
                tc.tile_pool(name="proj2p", bufs=2, space="PSUM") as proj2p,
            ):
                for ec in range(8):
                    fp = proj2p.tile([128, QSLICE], F32, space="PSUM",
                                     tag="p2")
                    for i, dp in enumerate((1, 3, 5, 7)):
                        nc.tensor.matmul(
                            fp[:], lhsT=wo[:, dp, ts(ec, 128)],
                            rhs=rwo[:, dp, :],
                            start=(i == 0), stop=(i == 3),
                        )
                    fin_sb = finp.tile([128, QSLICE], BF16, tag="fin")
                    nc.vector.tensor_tensor(fin_sb[:], fe[:, ec, :], fp[:],
                                            ADD)
                    nc.sync.dma_start(out=finT[ts(ec, 128), :], in_=fin_sb[:])

    nc.compile()
    return nc


def _host_prep(x, token_positions, W_qkv, W_o):
    bf16 = ml_dtypes.bfloat16
    xT = np.ascontiguousarray(np.transpose(x, (0, 2, 1))).astype(bf16)  # [B,D,S]

    # per-group W_qkv^T slices (Q rows pre-scaled by 1/sqrt(dk))
    wq = W_qkv[0 * D:1 * D] * np.float32(1.0 / np.sqrt(DK))
    wk = W_qkv[1 * D:2 * D]
    wv = W_qkv[2 * D:3 * D]
    wslices = []
    for g in range(GROUPS):
        rows = slice(g * HPC * DK, (g + 1) * HPC * DK)
        wsl = np.concatenate([wq[rows], wk[rows], wv[rows]], axis=0)  # [768, D]
        wslices.append(np.ascontiguousarray(wsl.T).astype(bf16))      # [D, 768]

    woT = np.ascontiguousarray(W_o.T).astype(bf16)                    # [D, D]

    idx = np.arange(DK // 2, dtype=np.float64)
    freqs = 1.0 / (THETA ** (2.0 * idx / DK))
    ang = np.arange(MAXPOS, dtype=np.float64)[:, None] * freqs[None, :]
    cstab = np.zeros((MAXPOS, 2 * DK), dtype=np.float32)
    cstab[:, 0:DK:2] = np.cos(ang)
    cstab[:, 1:DK:2] = np.cos(ang)
    cstab[:, DK::2] = np.sin(ang)
    cstab[:, DK + 1::2] = np.sin(ang)

    tri = (np.arange(128)[None, :] >= np.arange(128)[:, None]).astype(bf16)

    # pos transposed host-side: pos[r, c] = positions[128*c + r]
    posi = np.asarray(token_positions).astype(np.int32).reshape(B, NSC, 128)
    posi = np.ascontiguousarray(np.transpose(posi, (0, 2, 1)))  # [B,128,NSC]

    # wsel[r, j] = row of agout[h] holding peer (b, j)'s dims for my slice
    rr = np.arange(128)
    in_maps = []
    for c in range(N_CORES):
        b = c // GROUPS
        wsel = (512 * b + 128 * np.arange(GROUPS)[None, :]
                + rr[:, None]).astype(np.int32)
        in_maps.append({
            "xT": np.asarray(xT[b]),
            "wqkvT": wslices[c % GROUPS],
            "woT": woT,
            "cstab": cstab,
            "pos": posi[b],
            "tri": tri,
            "wsel": wsel,
        })
    return in_maps


def _assemble(results):
    out = np.empty((B, S, D), dtype=np.float32)
    for b in range(B):
        fullT = np.concatenate(
            [results[b * GROUPS + g]["finT"].astype(np.float32)
             for g in range(GROUPS)], axis=1)
        out[b] = fullT.T
    return out


_NC_CACHE = {}


def run(inputs, trace=False, **kw):
    if "nc" not in _NC_CACHE:
        _NC_CACHE["nc"] = _build()
    nc = _NC_CACHE["nc"]
    in_maps = _host_prep(**inputs)
    res = run_bass_kernel_spmd(
        nc, in_maps, core_ids=list(range(N_CORES)), trace=trace, **kw)
    return _assemble(res.results), res


def kernel(**inputs):
    out, _ = run(inputs, trace=False)
    return out


# revision 35
# speedup vs baseline: 1.4568x; 1.0481x over previous
"""Multi-head self-attention (RoPE, causal) on 8 trn2 NeuronCores.

Sharding: core c -> batch b = c // 4, head group g = c % 4 (4 heads each).
Each core:
  - projects Q,K,V for its batch / its 4 heads (token-major), applies RoPE
    via a gathered cos/sin table, transposes Q,K to [dk, S] layout,
  - computes scores^T = K^T-chunk x Q (k on partitions), exp on ACT,
    then out^T[d, q] accumulated fully in PSUM via V'-stationary matmuls
    where V' carries 64 appended ones-columns so PSUM rows 64:128 hold the
    softmax sums pre-broadcast; normalization is then just an ACT
    partition-shift copy + fast reciprocal + one multiply per q-block,
  - exchanges the head-sharded attention output with its batch peers via
    an 8-way AllToAll per head pair (cross-batch blocks duplicated as
    padding so the program stays SPMD-uniform), gathers its q-slice rows
    with a per-core index table, and runs the output projection in two
    passes so the first AllToAll's data is consumed during attention.
Host only reshapes/transposes/casts inputs and concatenates outputs.
"""

import sys

for _p in ("/opt/trn_rl_repo",):
    if _p not in sys.path:
        sys.path.append(_p)

import numpy as np
import ml_dtypes

import concourse.bass as bass
import concourse.mybir as mybir
import concourse.tile as tile
from concourse import bacc
from concourse.bass import ds, ts
from concourse.bass_utils import run_bass_kernel_spmd
from concourse.masks import make_identity

BF16 = mybir.dt.bfloat16
F32 = mybir.dt.float32
I32 = mybir.dt.int32

B, S, D = 2, 2048, 1024
H, DK = 16, 64
THETA = 10000.0
MAXPOS = 2048
N_CORES = 8
GROUPS = 4          # head groups (cores) per batch
HPC = H // GROUPS   # heads per core = 4
QKV_COLS = 3 * HPC * DK        # 768 per-core projection width
QK_COLS = 2 * HPC * DK         # 512 (Q then K)
NSC = S // 128                 # 16 token chunks
NQC = S // 512                 # 4 q column-chunks
QSLICE = S // GROUPS           # 512 output tokens per core
MUL = mybir.AluOpType.mult
ADD = mybir.AluOpType.add
SUB = mybir.AluOpType.subtract
COPY_F = mybir.ActivationFunctionType.Copy
EXP_F = mybir.ActivationFunctionType.Exp


def _build():
    nc = bacc.Bacc("TRN2", num_devices=N_CORES)

    xT = nc.dram_tensor("xT", [D, S], BF16, kind="ExternalInput")
    wqkvT = nc.dram_tensor("wqkvT", [D, QKV_COLS], BF16, kind="ExternalInput")
    woT = nc.dram_tensor("woT", [D, D], BF16, kind="ExternalInput")
    cstab = nc.dram_tensor("cstab", [MAXPOS, 2 * DK], F32, kind="ExternalInput")
    pos = nc.dram_tensor("pos", [128, NSC], I32, kind="ExternalInput")
    tri = nc.dram_tensor("tri", [128, 128], BF16, kind="ExternalInput")
    wsel = nc.dram_tensor("wsel", [128, 2 * GROUPS], I32, kind="ExternalInput")
    finT = nc.dram_tensor("finT", [D, QSLICE], BF16, kind="ExternalOutput")

    with tile.TileContext(nc) as tc:
        with (
            tc.tile_pool(name="const", bufs=1) as constp,
            tc.tile_pool(name="wts", bufs=1) as wtsp,
            tc.tile_pool(name="seq", bufs=1) as seqp,
            tc.tile_pool(name="attp", bufs=1) as attp,
            tc.tile_pool(name="dram", bufs=1, space="DRAM") as dramp,
        ):
            # ---------------- early DMAs: only what phase 1 needs ----------
            wt = wtsp.tile([128, 8, QKV_COLS], BF16)       # [dchunk][768]
            for k in range(8):
                nc.sync.dma_start(out=wt[:, k, 0:512],
                                  in_=wqkvT[ts(k, 128), 0:512])

            pidx = constp.tile([128, NSC], I32)
            nc.sync.dma_start(out=pidx[:], in_=pos[:])
            tri_t = constp.tile([128, 128], BF16)
            nc.sync.dma_start(out=tri_t[:], in_=tri[:])
            wsel_sb = constp.tile([128, 2 * GROUPS], I32)
            nc.sync.dma_start(out=wsel_sb[:], in_=wsel[:])
            ident = constp.tile([128, 128], BF16)
            make_identity(nc, ident[:])

            # persistent per-core tensors
            # Q^T zero-padded per head to full 128 contraction rows: head h
            # occupies rows (h%2)*64..+64, the other 64 rows stay zero so
            # scores matmuls contract over the full kt pair tile
            qt = seqp.tile([128, HPC, S], BF16)
            nc.vector.memset(qt[:], 0.0)
            kt = seqp.tile([128, 2, S], BF16)   # K^T  [pair, dk(2x64), k]
            # V + 64 ones cols per head: [k-token][chunk][head][v64|ones64]
            vv = seqp.tile([128, NSC, HPC, 2 * DK], BF16)
            nc.vector.memset(vv[:, :, :, DK:], 1.0)
            attT = attp.tile([128, 2, S], BF16)
            rwo = attp.tile([128, 8, QSLICE], BF16)
            fe = attp.tile([128, 8, QSLICE], F32)
            wo = wtsp.tile([128, 8, D], BF16)   # loaded late (see below)

            # ------------- projection + RoPE + transposes (scoped x) -------
            with (
                tc.tile_pool(name="xap", bufs=1) as xap,
                tc.tile_pool(name="ropet", bufs=2) as ropet,
                tc.tile_pool(name="pbig", bufs=2, space="PSUM") as pbig,
                tc.tile_pool(name="psmall", bufs=2, space="PSUM") as psmall,
            ):
                xa = xap.tile([128, 8, S], BF16)           # resident x^T
                for k in range(8):
                    nc.sync.dma_start(
                        out=xa[:, k, 0:512], in_=xT[ts(k, 128), 0:512])
                for k in range(8):
                    nc.sync.dma_start(out=wt[:, k, 512:768],
                                      in_=wqkvT[ts(k, 128), 512:768])
                for q in range(1, 4):
                    for k in range(8):
                        nc.sync.dma_start(
                            out=xa[:, k, ts(q, 512)],
                            in_=xT[ts(k, 128), ts(q, 512)])
                cs = xap.tile([128, NSC, 2 * DK], F32)     # cos/sin gather
                for c in range(NSC):
                    nc.gpsimd.indirect_dma_start(
                        out=cs[:, c, :],
                        out_offset=None,
                        in_=cstab[:],
                        in_offset=bass.IndirectOffsetOnAxis(
                            ap=pidx[:, c:c + 1], axis=0),
                    )
                # tiny dummy collective: absorbs the inter-core arrival skew
                # early (fully overlapped by phase 1) so the first real A2A
                # sees a short entry wait
                dumin = dramp.tile([8, 32], BF16, name="dumin")
                dumout = dramp.tile([8, 32], BF16, name="dumout")
                nc.gpsimd.collective_compute(
                    "AllToAll", mybir.AluOpType.bypass,
                    ins=[dumin[:]], outs=[dumout[:]],
                    replica_groups=[[0, 1, 2, 3, 4, 5, 6, 7]],
                )

                for sc in range(NSC):
                    ps = pbig.tile([128, QKV_COLS], F32, space="PSUM",
                                   tag="big")
                    for k in range(8):
                        nc.tensor.matmul(
                            ps[:, 0:512], lhsT=xa[:, k, ts(sc, 128)],
                            rhs=wt[:, k, 0:512],
                            start=(k == 0), stop=(k == 7),
                        )
                    for k in range(8):
                        nc.tensor.matmul(
                            ps[:, 512:768], lhsT=xa[:, k, ts(sc, 128)],
                            rhs=wt[:, k, 512:768],
                            start=(k == 0), stop=(k == 7),
                        )

                    # RoPE over the Q,K halves (cols 0:512), 8 blocks of 64
                    def qk_ap(off):
                        a = ps[:]
                        return bass.AP(a.tensor, a.offset + off,
                                       [a.ap[0], [DK, 2 * HPC], [2, DK // 2]])

                    def cs_ap(off):
                        a = cs[:, sc, :]
                        return bass.AP(a.tensor, a.offset + off,
                                       [a.ap[0], [0, 2 * HPC], [2, DK // 2]])

                    t1 = ropet.tile([128, 2 * HPC, DK // 2], F32, tag="t1")
                    t2 = ropet.tile([128, 2 * HPC, DK // 2], F32, tag="t2")
                    t3 = ropet.tile([128, 2 * HPC, DK // 2], F32, tag="t3")
                    t4 = ropet.tile([128, 2 * HPC, DK // 2], F32, tag="t4")
                    roped = ropet.tile([128, QK_COLS], BF16, tag="roped")

                    def roped_ap(off):
                        a = roped[:]
                        return bass.AP(a.tensor, a.offset + off,
                                       [a.ap[0], [DK, 2 * HPC], [2, DK // 2]])

                    nc.vector.tensor_tensor(t1[:], qk_ap(0), cs_ap(0), MUL)
                    nc.vector.tensor_tensor(t2[:], qk_ap(1), cs_ap(DK), MUL)
                    nc.vector.tensor_tensor(roped_ap(0), t1[:], t2[:], SUB)
                    nc.vector.tensor_tensor(t3[:], qk_ap(0), cs_ap(DK), MUL)
                    nc.vector.tensor_tensor(t4[:], qk_ap(1), cs_ap(0), MUL)
                    nc.vector.tensor_tensor(roped_ap(1), t3[:], t4[:], ADD)

                    # V columns (ones cols already set) -- on ACT
                    nc.scalar.activation(
                        vv[:, sc, :, 0:DK],
                        ps[:, 512:768].rearrange("p (h e) -> p h e", h=HPC),
                        COPY_F,
                    )

                    # transpose the 4 roped q/k 128-col blocks -> qt/kt
                    for t in range(4):
                        tp = psmall.tile([128, 128], BF16, space="PSUM",
                                         tag="small")
                        nc.tensor.transpose(tp[:], roped[:, ts(t, 128)],
                                            ident[:])
                        if t < 2:   # q pair t -> padded per-head slots
                            nc.scalar.activation(
                                qt[0:DK, 2 * t, ts(sc, 128)], tp[0:DK, :],
                                COPY_F)
                            nc.scalar.activation(
                                qt[ds(DK, DK), 2 * t + 1, ts(sc, 128)],
                                tp[ds(DK, DK), :], COPY_F)
                        else:
                            nc.scalar.activation(kt[:, t % 2, ts(sc, 128)],
                                                 tp[:], COPY_F)

                # late weight load: queued behind phase-1 DMAs on purpose
                for k in range(8):
                    nc.sync.dma_start(out=wo[:, k, :], in_=woT[ts(k, 128), :])

            # ------------- attention + A2A: pair 0 whole, pair 1 per head --
            agin = [dramp.tile([8 * 128, QSLICE], BF16, name="agin0")]
            agout = [dramp.tile([8 * 128, QSLICE], BF16, name="agout0")]
            aginh = {h: dramp.tile([8 * DK, QSLICE], BF16, name=f"aginh{h}")
                     for h in (2, 3)}
            agouth = {h: dramp.tile([8 * DK, QSLICE], BF16, name=f"agouth{h}")
                      for h in (2, 3)}

            with (
                tc.tile_pool(name="etp", bufs=3) as etp,
                tc.tile_pool(name="rsbp", bufs=2) as rsbp,
                tc.tile_pool(name="scoresp", bufs=2, space="PSUM") as scoresp,
                tc.tile_pool(name="vaccp", bufs=4, space="PSUM") as vaccp,
            ):
                def emit_sblock(h, j, et, qh):
                    # scores + one exp for k-chunk j, 1024-col window qh
                    pr, hf = h // 2, (h % 2) * DK
                    a0 = max(1024 * qh, 128 * j)
                    sp = scoresp.tile([128, 1024], F32, space="PSUM",
                                      tag="sc", name=f"sp{h}_{j}_{qh}")
                    for qq in (1024 * qh, 1024 * qh + 512):
                        a = max(a0, qq)
                        w = qq + 512 - a
                        if w <= 0:
                            continue
                        nc.tensor.matmul(
                            sp[:, ds(a - 1024 * qh, w)],
                            lhsT=kt[:, pr, ts(j, 128)],
                            rhs=qt[:, h, ds(a, w)],
                            start=True, stop=True,
                        )
                    nc.scalar.activation(
                        et[:, ds(a0, 1024 * (qh + 1) - a0)],
                        sp[:, ds(a0 - 1024 * qh, 1024 * (qh + 1) - a0)],
                        EXP_F)
                    if qh == j // 8:   # diagonal block: mask q < k to 0
                        nc.vector.tensor_tensor(
                            et[:, ts(j, 128)], et[:, ts(j, 128)], tri_t[:],
                            MUL)

                def emit_vblock(h, j, et, vacc, qb):
                    a = max(512 * qb, 128 * j)
                    w = 512 * (qb + 1) - a
                    nc.tensor.matmul(
                        vacc[qb][:, ds(a - 512 * qb, w)],
                        lhsT=vv[:, j, h, :],
                        rhs=et[:, ds(a, w)],
                        start=(j == 0), stop=(j == 4 * qb + 3),
                    )

                def emit_normalize(h, vacc):
                    pr, hf = h // 2, (h % 2) * DK
                    for qc in range(NQC):
                        rsb = rsbp.tile([DK, 512], F32, tag="rsb")
                        nc.vector.tensor_copy(rsb[:], vacc[qc][ds(DK, DK), :])
                        nc.vector.reciprocal_approx_fast(rsb[:], rsb[:])
                        nc.vector.tensor_tensor(
                            attT[ds(hf, DK), pr, ts(qc, 512)],
                            vacc[qc][0:DK, :], rsb[:], MUL)
                        if h == 1:       # pair slice done -> stage for A2A
                            for bh in range(2):
                                nc.sync.dma_start(
                                    out=agin[0][ds((4 * bh + qc) * 128, 128), :],
                                    in_=attT[:, 0, ts(qc, 512)],
                                )
                        elif h >= 2:     # pair 1 staged per head
                            for bh in range(2):
                                nc.sync.dma_start(
                                    out=aginh[h][ds((4 * bh + qc) * DK, DK), :],
                                    in_=attT[ds(hf, DK), 1, ts(qc, 512)],
                                )

                def emit_a2a0():
                    nc.gpsimd.collective_compute(
                        "AllToAll", mybir.AluOpType.bypass,
                        ins=[agin[0][:]], outs=[agout[0][:]],
                        replica_groups=[[0, 1, 2, 3, 4, 5, 6, 7]],
                    )
                    for j in range(GROUPS):
                        nc.gpsimd.indirect_dma_start(
                            out=rwo[:, 2 * j, :],
                            out_offset=None,
                            in_=agout[0][:],
                            in_offset=bass.IndirectOffsetOnAxis(
                                ap=wsel_sb[:, j:j + 1], axis=0),
                        )

                def emit_a2a_head(h):
                    # head h of pair 1 -> rwo rows 64*(h%2) of odd dps
                    nc.gpsimd.collective_compute(
                        "AllToAll", mybir.AluOpType.bypass,
                        ins=[aginh[h][:]], outs=[agouth[h][:]],
                        replica_groups=[[0, 1, 2, 3, 4, 5, 6, 7]],
                    )
                    for j in range(GROUPS):
                        nc.gpsimd.indirect_dma_start(
                            out=rwo[ds((h % 2) * DK, DK), 2 * j + 1, :],
                            out_offset=None,
                            in_=agouth[h][:],
                            in_offset=bass.IndirectOffsetOnAxis(
                                ap=wsel_sb[0:DK, 4 + j:5 + j], axis=0),
                        )

                def emit_proj1(ec):
                    # shares the scores psum ring (same shape, half used)
                    fp = scoresp.tile([128, 1024], F32, space="PSUM",
                                      tag="sc", name=f"fp{ec}")
                    for i, dp in enumerate((0, 2, 4, 6)):
                        nc.tensor.matmul(
                            fp[:, 0:QSLICE], lhsT=wo[:, dp, ts(ec, 128)],
                            rhs=rwo[:, dp, :],
                            start=(i == 0), stop=(i == 3),
                        )
                    nc.vector.tensor_copy(fe[:, ec, :], fp[:, 0:QSLICE])

                for h in range(HPC):
                    vacc = [vaccp.tile([128, 512], F32, space="PSUM",
                                       tag="va", name=f"va{h}_{q}")
                            for q in range(NQC)]
                    ets = {}
                    for j in range(NSC):
                        et = etp.tile([128, S], BF16, tag="exp",
                                      name=f"et{h}_{j}")
                        ets[j] = et
                        # interleave: scores/exp for (j, qh) two chunks ahead
                        # of the V-matmuls for (j-2, qb) so ACT stays ahead
                        jv = j - 2
                        vqbs = (list(range(jv // 4, NQC)) if jv >= 0 else [])
                        for qh in range(j // 8, 2):
                            emit_sblock(h, j, et, qh)
                            while vqbs and (vqbs[0] <= 2 * qh + 1 or qh == 1):
                                emit_vblock(h, jv, ets[jv], vacc,
                                            vqbs.pop(0))
                        ets.pop(j - 2, None)
                    for jv in (NSC - 2, NSC - 1):
                        for qb in range(jv // 4, NQC):
                            emit_vblock(h, jv, ets[jv], vacc, qb)
                    emit_normalize(h, vacc)
                    if h == 1:
                        emit_a2a0()
                    elif h >= 2:
                        emit_a2a_head(h)
                # proj pass 1 runs during the last A2A's flight
                for ec in range(8):
                    emit_proj1(ec)

            # ------------- output projection pass 2 ------------------------
            with (
                tc.tile_pool(name="finp", bufs=2) as finp,
                tc.tile_pool(name="proj2p", bufs=2, space="PSUM") as proj2p,
            ):
                for ec in range(8):
                    fp = proj2p.tile([128, QSLICE], F32, space="PSUM",
                                     tag="p2")
                    for i, dp in enumerate((1, 3, 5, 7)):
                        nc.tensor.matmul(
                            fp[:], lhsT=wo[:, dp, ts(ec, 128)],
                            rhs=rwo[:, dp, :],
                            start=(i == 0), stop=(i == 3),
                        )
                    fin_sb = finp.tile([128, QSLICE], BF16, tag="fin")
                    nc.vector.tensor_tensor(fin_sb[:], fe[:, ec, :], fp[:],
                                            ADD)
                    nc.sync.dma_start(out=finT[ts(ec, 128), :], in_=fin_sb[:])

    nc.compile()
    return nc


def _host_prep(x, token_positions, W_qkv, W_o):
    bf16 = ml_dtypes.bfloat16
    xT = np.ascontiguousarray(np.transpose(x, (0, 2, 1))).astype(bf16)  # [B,D,S]

    # per-group W_qkv^T slices (Q rows pre-scaled by 1/sqrt(dk))
    wq = W_qkv[0 * D:1 * D] * np.float32(1.0 / np.sqrt(DK))
    wk = W_qkv[1 * D:2 * D]
    wv = W_qkv[2 * D:3 * D]
    wslices = []
    for g in range(GROUPS):
        rows = slice(g * HPC * DK, (g + 1) * HPC * DK)
        wsl = np.concatenate([wq[rows], wk[rows], wv[rows]], axis=0)  # [768, D]
        wslices.append(np.ascontiguousarray(wsl.T).astype(bf16))      # [D, 768]

    woT = np.ascontiguousarray(W_o.T).astype(bf16)                    # [D, D]

    idx = np.arange(DK // 2, dtype=np.float64)
    freqs = 1.0 / (THETA ** (2.0 * idx / DK))
    ang = np.arange(MAXPOS, dtype=np.float64)[:, None] * freqs[None, :]
    cstab = np.zeros((MAXPOS, 2 * DK), dtype=np.float32)
    cstab[:, 0:DK:2] = np.cos(ang)
    cstab[:, 1:DK:2] = np.cos(ang)
    cstab[:, DK::2] = np.sin(ang)
    cstab[:, DK + 1::2] = np.sin(ang)

    tri = (np.arange(128)[None, :] >= np.arange(128)[:, None]).astype(bf16)

    # pos transposed host-side: pos[r, c] = positions[128*c + r]
    posi = np.asarray(token_positions).astype(np.int32).reshape(B, NSC, 128)
    posi = np.ascontiguousarray(np.transpose(posi, (0, 2, 1)))  # [B,128,NSC]

    # wsel[r, j] = row of agout[h] holding peer (b, j)'s dims for my slice
    rr = np.arange(128)
    in_maps = []
    for c in range(N_CORES):
        b = c // GROUPS
        wsel = np.zeros((128, 2 * GROUPS), dtype=np.int32)
        wsel[:, 0:GROUPS] = (512 * b + 128 * np.arange(GROUPS)[None, :]
                             + rr[:, None])
        wsel[0:DK, GROUPS:] = (256 * b + DK * np.arange(GROUPS)[None, :]
                               + rr[0:DK, None])
        wsel = wsel.astype(np.int32)
        in_maps.append({
            "xT": np.asarray(xT[b]),
            "wqkvT": wslices[c % GROUPS],
            "woT": woT,
            "cstab": cstab,
            "pos": posi[b],
            "tri": tri,
            "wsel": wsel,
        })
    return in_maps


def _assemble(results):
    out = np.empty((B, S, D), dtype=np.float32)
    for b in range(B):
        fullT = np.concatenate(
            [results[b * GROUPS + g]["finT"].astype(np.float32)
             for g in range(GROUPS)], axis=1)
        out[b] = fullT.T
    return out


_NC_CACHE = {}


def run(inputs, trace=False, **kw):
    if "nc" not in _NC_CACHE:
        _NC_CACHE["nc"] = _build()
    nc = _NC_CACHE["nc"]
    in_maps = _host_prep(**inputs)
    res = run_bass_kernel_spmd(
        nc, in_maps, core_ids=list(range(N_CORES)), trace=trace, **kw)
    return _assemble(res.results), res


def kernel(**inputs):
    out, _ = run(inputs, trace=False)
    return out
